# revision 39
# baseline (speedup 1.0000x reference)
"""ChebyNet (K=1) dual-branch MLP + BN kernel for 8 Trainium2 NeuronCores.

Network (per reference):
  branch b in {1,2}:  h = relu(BN(x_b @ W1_b)) ; h = relu(BN(h @ W2_b)) ; f_b = h @ Wf_b + bf_b
  out = relu(concat(f_1, f_2) @ Wh1 + bh1) @ Wh2 + bh2

ChebConv with K=1 ignores edge_index/edge_weight entirely.  Training-mode
BatchNorm over the node axis makes the linear-layer biases b1/b2 cancel
exactly, so they are never loaded.

Key restructurings vs the direct form:
  * Wf_b and Wh1 compose linearly (no nonlinearity between them), so the
    host folds M_b = Wf_b @ Wh1[b-half] and b' = bf_1 @ Wh1a + bf_2 @ Wh1b
    + bh1.  The Lf layer and the concat disappear: t = relu(h2_1 @ M_1 +
    h2_2 @ M_2 + b'), out = t @ Wh2 + bh2.
  * The host pre-transposes x into feature-major xT (bf16) and a
    partition-major node layout xg for the Gram pass, so the kernel never
    runs PE transposes.
  * Layer-1 BN stats use the Gram identity: sumsq(pre1) = diag(W1^T (X^T X)
    W1), sum(pre1) = W1^T (X^T 1).  X^T 1 comes from the same Gram
    stationary tiles with a [128,4] ones moving operand (nearly free).
  * Pass 2 computes branch-0 L1 for all chunks before BN1 stats arrive,
    parking pre1 as bf16 ("loop A"); the BN1+relu+L2 pass ("loop B") then
    never stalls on the first AllReduce.

Sharding: nodes (axis 0) split across 8 cores, 12500 each, zero-padded to
12544 = 98*128.  Weights replicated.  BN batch stats are combined with an
AllReduce(add) of per-core (sum, sumsq); the four collectives are
interleaved so each hides under the next phase's compute.
"""

import os

os.environ.setdefault("JAX_PLATFORMS", "axon,cpu")

import numpy as np

import concourse.bacc as bacc
import concourse.mybir as mybir
import concourse.tile as tile
from concourse import bass_utils
from concourse.bass import ts

F32 = mybir.dt.float32
F32R = mybir.dt.float32r
BF16 = mybir.dt.bfloat16
AF = mybir.ActivationFunctionType
ALU = mybir.AluOpType

NTOT = 100000          # true node count
NCORES = 8
NSH = NTOT // NCORES   # 12500 true nodes per core
NP = 12544             # padded per-core nodes (= 98 * 128)
T = 512                # node-chunk size (free dim of matmuls / PSUM bank)
CHUNKS = [(i * T, T) for i in range(NP // T)] + ([(NP - NP % T, NP % T)] if NP % T else [])
C = len(CHUNKS)
NSUB = NP // 128       # 98 gram sub-tiles
GGRP = 25              # max gram sub-tiles per DMA
GRAM_GROUPS = [(0, 9), (9, 22), (31, 22), (53, 22), (75, 23)]
XSLAB = [(0, 3072), (3072, 3072), (6144, 3072), (9216, NP - 9216)]
PAD0 = NSH - (NP - (NP % T or T))  # first padded column inside last chunk (212)
EPS = 1e-5

_CACHE = {}


def _build_program():
    nc = bacc.Bacc("TRN2", target_bir_lowering=False, debug=False,
                   num_devices=NCORES)

    # ---- kernel I/O -----------------------------------------------------
    xT_d = [nc.dram_tensor(f"xT_{b}", [128, NP], BF16, kind="ExternalInput")
            for b in range(2)]
    xg_d = [nc.dram_tensor(f"xg_{b}", [128, NSUB, 128], BF16,
                           kind="ExternalInput") for b in range(2)]
    w1_d = [nc.dram_tensor(f"W1_{b}", [128, 512], BF16, kind="ExternalInput")
            for b in range(2)]
    w2_d = [nc.dram_tensor(f"W2_{b}", [128, 4, 512], BF16,
                           kind="ExternalInput") for b in range(2)]
    m_d = [nc.dram_tensor(f"M_{b}", [128, 4, 512], BF16, kind="ExternalInput")
           for b in range(2)]
    wh2_d = nc.dram_tensor("WH2", [128, 4, 10], BF16, kind="ExternalInput")
    bp_d = nc.dram_tensor("BP", [128, 4], F32, kind="ExternalInput")
    bh2_d = nc.dram_tensor("BH2", [10, 1], F32, kind="ExternalInput")
    g_d = [[nc.dram_tensor(f"g{l}_{b}", [128, 4], F32, kind="ExternalInput")
            for b in range(2)] for l in range(2)]
    be_d = [[nc.dram_tensor(f"be{l}_{b}", [128, 4], F32, kind="ExternalInput")
             for b in range(2)] for l in range(2)]
    aux_d = nc.dram_tensor("AUX", [128, 5], F32, kind="ExternalInput")
    outd = nc.dram_tensor("OUT", [10, NP], F32, kind="ExternalOutput")

    # ---- DRAM scratch ---------------------------------------------------
    spill = nc.dram_tensor("pre2_spill", [128, 2, 4, NP], BF16)
    cc_in = [[nc.dram_tensor(f"cc{l}{b}_in", [128, 4, 2], F32) for b in range(2)]
             for l in range(2)]
    cc_out = [[nc.dram_tensor(f"cc{l}{b}_out", [NCORES, 128, 4, 2], F32,
                              addr_space="Shared") for b in range(2)]
              for l in range(2)]

    with tile.TileContext(nc) as tc:
        with (
            tc.tile_pool(name="wpool", bufs=1) as wp,
            tc.tile_pool(name="stat", bufs=1) as stat,
        ):
            # ---- constants via one small DMA (memset ISA is rejected by
            # the walrus codegen) ------------------------------------------
            aux_sb = wp.tile([128, 5], F32, name="aux_sb")
            nc.scalar.dma_start(aux_sb[:], aux_d[:, :])
            eps_t = aux_sb[:, 4:5]
            ones_bf = wp.tile([128, 4], BF16, name="ones_bf")
            nc.vector.tensor_copy(ones_bf[:], aux_sb[:, 0:4])
            ones_r = wp.tile([128, 4], F32R, name="ones_r")
            nc.vector.tensor_copy(ones_r[:], aux_sb[:, 0:4])

            # W1 now (pass-1 projection needs it); the rest deferred.
            w1_bf, w1_r = [], []
            for b in range(2):
                w1b = wp.tile([128, 512], BF16, name=f"w1b_{b}")
                nc.scalar.dma_start(w1b[:], w1_d[b][:, :])
                w1r = wp.tile([128, 512], F32R, name=f"w1r_{b}")
                nc.vector.tensor_copy(w1r[:], w1b[:])
                w1_bf.append(w1b)
                w1_r.append(w1r)

            w2_t = [wp.tile([128, 4, 512], BF16, name=f"w2_{b}") for b in range(2)]
            m_t = [wp.tile([128, 4, 512], BF16, name=f"m_{b}") for b in range(2)]
            wh2_t = wp.tile([128, 4, 10], BF16, name="wh2_t")
            bp_sb = wp.tile([128, 4], F32, name="bp_sb")
            bh2_sb = wp.tile([10, 1], F32, name="bh2_sb")
            g_sb = [stat.tile([128, 2, 4], F32, name=f"g_sb{l}") for l in range(2)]
            be_sb = [stat.tile([128, 2, 4], F32, name=f"be_sb{l}") for l in range(2)]

            def load_pass2_weights():
                for l in range(2):
                    for b in range(2):
                        nc.scalar.dma_start(g_sb[l][:, b, :], g_d[l][b][:, :])
                        nc.scalar.dma_start(be_sb[l][:, b, :], be_d[l][b][:, :])

            def load_pass3_weights():
                for b in range(2):
                    nc.scalar.dma_start(m_t[b][:], m_d[b][:, :, :])
                nc.scalar.dma_start(wh2_t[:], wh2_d[:, :, :])
                nc.scalar.dma_start(bp_sb[:], bp_d[:, :])
                nc.scalar.dma_start(bh2_sb[:], bh2_d[:, :])

            st2 = stat.tile([128, 2, 4, C, 6], F32, name="st2")
            pay = [[stat.tile([128, 4, 2], F32, name=f"pay{l}{b}")
                    for b in range(2)] for l in range(2)]
            scale_t = [stat.tile([128, 2, 4], F32, name=f"scale{l}") for l in range(2)]
            shift_t = [stat.tile([128, 2, 4], F32, name=f"shift{l}") for l in range(2)]

            gl8_tiles = {}

            def issue_payload(l, b):
                # payload on SP at a chosen emission point: its wait-for-
                # payload intentionally head-of-line-throttles SP so the
                # tiny cc_in transfer is NOT queued behind bulk loads in the
                # single-slot DMA-engine FIFO.  AllGather + local sum: the
                # collective model charges AllReduce 1.875x the gather time.
                nc.sync.dma_start(cc_in[l][b][:, :, :], pay[l][b][:])

            def issue_collective(l, b):
                nc.gpsimd.collective_compute(
                    "AllGather", mybir.AluOpType.bypass,
                    replica_groups=[list(range(NCORES))],
                    ins=[cc_in[l][b].ap().opt()], outs=[cc_out[l][b].ap().opt()],
                )
                # gather-load immediately after its collective in the Pool
                # FIFO: runs as soon as the AllGather lands
                gl8 = stat.tile([128, NCORES, 4, 2], F32, tag=f"gl8{l}{b}",
                                name=f"gl8{l}{b}")
                nc.gpsimd.dma_start(
                    gl8[:], cc_out[l][b].ap().rearrange("n p m s -> p n m s"))
                gl8_tiles[(l, b)] = gl8

            def issue_allreduce(l, b):
                issue_payload(l, b)
                issue_collective(l, b)

            def finish_stats(l, b):
                """gathered sums -> scale_t[l][:, b, :], shift_t[l][:, b, :]."""
                gl8 = gl8_tiles.pop((l, b))
                gl = stat.tile([128, 4, 2], F32, tag="gl", name=f"gl{l}{b}")
                nc.vector.tensor_add(gl[:], gl8[:, 0, :, :], gl8[:, 1, :, :])
                for n in range(2, NCORES):
                    nc.vector.tensor_add(gl[:], gl[:], gl8[:, n, :, :])
                mu = stat.tile([128, 4], F32, tag="mu", name=f"mu{l}{b}")
                var = stat.tile([128, 4], F32, tag="var", name=f"var{l}{b}")
                tmp = stat.tile([128, 4], F32, tag="tmpf", name=f"tmp{l}{b}")
                nc.vector.tensor_scalar_mul(mu[:], gl[:, :, 0], 1.0 / NTOT)
                nc.vector.tensor_scalar_mul(var[:], gl[:, :, 1], 1.0 / NTOT)
                nc.vector.tensor_mul(tmp[:], mu[:], mu[:])
                nc.vector.tensor_sub(var[:], var[:], tmp[:])
                nc.scalar.activation(var[:], var[:], AF.Sqrt, bias=eps_t)
                nc.vector.reciprocal(var[:], var[:])
                nc.vector.tensor_mul(scale_t[l][:, b, :], g_sb[l][:, b, :], var[:])
                nc.vector.tensor_mul(tmp[:], mu[:], scale_t[l][:, b, :])
                nc.vector.tensor_sub(shift_t[l][:, b, :], be_sb[l][:, b, :], tmp[:])

            # ================= pass 1: Gram + xsum -> BN1 stats ==========
            # explicit pool lifetimes: pass-3 PSUM (ps_t/ps_o) must reuse the
            # banks of the PASS-1 pools (drained by ~30us), not pass-2's --
            # otherwise pass-3's prologue matmuls serialize behind the whole
            # of pass 2 on the pool-drain barrier.
            ps_p1 = tc.alloc_tile_pool(name="ps_p1", bufs=2, space="PSUM")
            xtp0 = tc.alloc_tile_pool(name="xtp0", bufs=1)
            parkp = tc.alloc_tile_pool(name="parkp", bufs=1)
            w2p = tc.alloc_tile_pool(name="w2p", bufs=2)
            g1p = tc.alloc_tile_pool(name="g1p", bufs=1)
            ps_g = tc.alloc_tile_pool(name="ps_g", bufs=1, space="PSUM")
            ps_xs = tc.alloc_tile_pool(name="ps_xs", bufs=1, space="PSUM")
            ps_pj = tc.alloc_tile_pool(name="ps_pj", bufs=1, space="PSUM")
            if True:
                def gram_load(b, gi):
                    j0, gsz = GRAM_GROUPS[gi]
                    xgt = g1p.tile([128, GGRP, 128], BF16, tag="xg",
                                   bufs=3, name=f"xg_{b}_{j0}")
                    nc.sync.dma_start(xgt[:, :gsz, :],
                                      xg_d[b][:, j0:j0 + gsz, :])
                    return xgt

                def gram_mms(b, gi, xgt, g_ps, xs_ps):
                    j0, gsz = GRAM_GROUPS[gi]
                    for j in range(gsz):
                        si = j0 + j
                        nc.tensor.matmul(g_ps[:], xgt[:, j, :], xgt[:, j, :],
                                         start=(si == 0), stop=(si == NSUB - 1))
                        nc.tensor.matmul(xs_ps[:], xgt[:, j, :], ones_bf[:],
                                         start=(si == 0), stop=(si == NSUB - 1))

                def gram_psum(b):
                    g_ps = ps_g.tile([128, 128], F32, tag="G", name=f"G_{b}")
                    xs_ps = ps_xs.tile([128, 4], F32, tag="XS", name=f"XS_{b}")
                    return g_ps, xs_ps

                def proj_a(b, g_ps, xs_ps):
                    g_sbuf = g1p.tile([128, 128], F32R, tag="gsb", name=f"gsb_{b}")
                    nc.vector.tensor_copy(g_sbuf[:], g_ps[:])
                    xsum_r = g1p.tile([128, 4], F32R, tag="xsumr", name=f"xsumr_{b}")
                    nc.vector.tensor_copy(xsum_r[:], xs_ps[:])
                    mm1 = ps_pj.tile([128, 512], F32, tag="pj", name=f"mm1_{b}")
                    nc.tensor.matmul(mm1[:], g_sbuf[:], w1_r[b][:], start=True,
                                     stop=True)
                    mm1_sb = g1p.tile([128, 512], F32R, tag="mm1sb",
                                      name=f"mm1sb_{b}")
                    nc.vector.tensor_copy(mm1_sb[:], mm1[:])
                    prod = g1p.tile([128, 512], F32R, tag="prod", name=f"prod_{b}")
                    nc.vector.tensor_mul(prod[:], w1_r[b][:], mm1_sb[:])
                    return xsum_r, prod

                def proj_b(b, xsum_r, prod):
                    # all 8 tiny matmuls land in slices of one PSUM bank so
                    # the in-order PE queue never waits on the DVE copies
                    pj = ps_pj.tile([128, 4, 2, 4], F32, tag="pj2",
                                    name=f"pj2_{b}")
                    for m in range(4):
                        nc.tensor.matmul(pj[:, m, 0, :],
                                         w1_r[b][:, ts(m, 128)], xsum_r[:],
                                         start=True, stop=True)
                        nc.tensor.matmul(pj[:, m, 1, :],
                                         prod[:, ts(m, 128)], ones_r[:],
                                         start=True, stop=True)
                    nc.vector.tensor_copy(pay[0][b][:, :, :], pj[:, :, :, 0])

                # branch-0 input (xT) lives resident in 4 slab tiles whose
                # loads interleave with the gram loads: few big DMA
                # instructions (the SP->HWDGE issue rate is ~1.3us/instr)
                xt0_slabs = [xtp0.tile([128, sz], BF16, tag=f"xts_{i}",
                                       name=f"xts_{i}")
                             for i, (s0, sz) in enumerate(XSLAB)]

                def emit_xts(i):
                    s0, sz = XSLAB[i]
                    nc.sync.dma_start(xt0_slabs[i][:], xT_d[0][:, s0:s0 + sz])

                def xt0_ap(c):
                    c0, tc_sz = CHUNKS[c]
                    for i, (s0, sz) in enumerate(XSLAB):
                        if s0 <= c0 and c0 + tc_sz <= s0 + sz:
                            return xt0_slabs[i][:, c0 - s0:c0 - s0 + tc_sz]
                    raise AssertionError(c)

                # ---- pass-2 b0 loop A: L1 + park (stats-independent),
                # interleaved into pass-1 to keep the in-order PE queue fed
                PARK_C = 12  # chunks parked ahead of the first AllGather
                park_tiles = {}

                def parkA(c):
                    c0, tc_sz = CHUNKS[c]
                    park = parkp.tile([128, 4, tc_sz], BF16, tag=f"park_{c}",
                                      name=f"park_{c}")
                    park_tiles[c] = park
                    for m in range(4):
                        pp = ps_p1.tile([128, tc_sz], F32, tag="p1",
                                        name=f"p1a_{c}_{m}")
                        nc.tensor.matmul(pp[:], w1_bf[0][:, ts(m, 128)],
                                         xt0_ap(c), start=True, stop=True)
                        if m % 2 == 0:
                            nc.scalar.copy(park[:, m, :], pp[:])
                        else:
                            nc.vector.tensor_copy(park[:, m, :], pp[:])

                g0, xs0 = gram_psum(0)
                xg0 = [gram_load(0, i) for i in range(len(GRAM_GROUPS))]
                for i in range(len(GRAM_GROUPS)):
                    gram_mms(0, i, xg0[i], g0, xs0)
                xr0, pr0 = proj_a(0, g0, xs0)
                proj_b(0, xr0, pr0)
                emit_xts(0)
                # SP stalls here on the layer-1 payload: the DMA queue holds
                # only the loads needed before the collective lands
                issue_payload(0, 0)
                issue_collective(0, 0)
                g1, xs1 = gram_psum(1)
                xg10 = gram_load(1, 0)
                xg11 = gram_load(1, 1)
                emit_xts(1)
                for c in range(0, 3):
                    parkA(c)
                xg12 = gram_load(1, 2)
                for c in range(3, 6):
                    parkA(c)
                gram_mms(1, 0, xg10, g1, xs1)
                gram_mms(1, 1, xg11, g1, xs1)
                xg13 = gram_load(1, 3)
                for c in range(6, 9):
                    parkA(c)
                gram_mms(1, 2, xg12, g1, xs1)
                # W2 via the ACT queue after several park copies: enters the
                # DMA pool late enough not to delay the layer-1 payload
                for b in range(2):
                    nc.scalar.dma_start(w2_t[b][:], w2_d[b][:, :, :])
                xg14 = gram_load(1, 4)
                for c in range(9, PARK_C):
                    parkA(c)
                gram_mms(1, 3, xg13, g1, xs1)
                gram_mms(1, 4, xg14, g1, xs1)
                xr1, pr1 = proj_a(1, g1, xs1)
                proj_b(1, xr1, pr1)
                issue_payload(0, 1)
                issue_collective(0, 1)
                emit_xts(2)
                emit_xts(3)
                load_pass2_weights()
                # prefetch branch-1 xT chunks so pass-2-b1 doesn't wait on
                # the loop-B spill queue to issue its first loads
                xt1_pre = {}
                for c in range(3):
                    c0, tc_sz = CHUNKS[c]
                    xt = xtp0.tile([128, tc_sz], BF16, tag="xt1",
                                   name=f"xt1_{c}", bufs=8)
                    nc.sync.dma_start(xt[:], xT_d[1][:, c0:c0 + tc_sz])
                    xt1_pre[c] = xt

                # pass-1 pools drain by ~30us; their PSUM banks become the
                # pass-3 accumulation banks (no dependency on pass-2 pools)
                ps_pj.release()
                ps_xs.release()
                ps_g.release()
                g1p.release()
                ps_p2 = tc.alloc_tile_pool(name="ps_p2", bufs=3, space="PSUM")
                ps_t = tc.alloc_tile_pool(name="ps_t", bufs=2, space="PSUM",
                                          side="right")
                ps_o = tc.alloc_tile_pool(name="ps_o", bufs=1, space="PSUM",
                                          side="right")

                # ---- pass-2 b0 loop B: BN1 -> L2 -> stats/spill ---------
                finish_stats(0, 0)
                for c, (c0, tc_sz) in enumerate(CHUNKS):
                    h1 = w2p.tile([128, 4, tc_sz], BF16, tag="h1",
                                  name=f"h1_{c}_0")
                    if c < PARK_C:
                        park = park_tiles.pop(c)
                        for m in range(4):
                            nc.scalar.activation(
                                h1[:, m, :], park[:, m, :], AF.Relu,
                                bias=shift_t[0][:, 0, m:m + 1],
                                scale=scale_t[0][:, 0, m:m + 1])
                    else:
                        for m in range(4):
                            pp = ps_p1.tile([128, tc_sz], F32, tag="p1",
                                            name=f"p1a_{c}_{m}")
                            nc.tensor.matmul(pp[:], w1_bf[0][:, ts(m, 128)],
                                             xt0_ap(c), start=True,
                                             stop=True)
                            nc.scalar.activation(
                                h1[:, m, :], pp[:], AF.Relu,
                                bias=shift_t[0][:, 0, m:m + 1],
                                scale=scale_t[0][:, 0, m:m + 1])
                    if c == C - 1:
                        nc.scalar.mul(h1[:, :, PAD0:], h1[:, :, PAD0:], 0.0)
                    spl = w2p.tile([128, 4, tc_sz], BF16, tag="spl",
                                   name=f"spl_{c}_0", bufs=4)
                    for m in range(4):
                        pq = ps_p2.tile([128, tc_sz], F32, tag="p2",
                                        name=f"p2_{c}_0_{m}")
                        for k in range(4):
                            nc.tensor.matmul(pq[:], w2_t[0][:, k, ts(m, 128)],
                                             h1[:, k, :],
                                             start=(k == 0), stop=(k == 3))
                        if m % 2 == 0 and c < 22:
                            nc.scalar.copy(spl[:, m, :], pq[:])
                        else:
                            nc.vector.tensor_copy(spl[:, m, :], pq[:])
                    for m in range(4):
                        nc.vector.bn_stats(st2[:, 0, m, c, :], spl[:, m, :])
                    nc.sync.dma_start(spill[:, 0, :, c0:c0 + tc_sz], spl[:])

                agg = stat.tile([128, 4, 2], F32, tag="agg", name="agg_0")
                for m in range(4):
                    nc.vector.bn_aggr(agg[:, m, :], st2[:, 0, m, :, :])
                tmp2 = stat.tile([128, 4], F32, tag="tmp2", name="tmp2_0")
                nc.vector.tensor_scalar_mul(pay[1][0][:, :, 0], agg[:, :, 0],
                                            float(NP))
                nc.vector.tensor_mul(tmp2[:], agg[:, :, 0], agg[:, :, 0])
                nc.vector.tensor_add(tmp2[:], tmp2[:], agg[:, :, 1])
                nc.vector.tensor_scalar_mul(pay[1][0][:, :, 1], tmp2[:],
                                            float(NP))
                issue_allreduce(1, 0)

                pf3_tiles = {}
                for c in range(5):
                    c0, tc_sz = CHUNKS[c]
                    pf = stat.tile([128, 4, tc_sz], BF16, tag=f"pf3_{c}",
                                   name=f"pf3_{c}")
                    nc.sync.dma_start(pf[:], spill[:, 0, :, c0:c0 + tc_sz])
                    pf3_tiles[c] = pf
                h2p_tiles = {}

                # ---- pass-2 b1 (direct PSUM path; stats(0,1) long ready) -
                finish_stats(0, 1)
                load_pass3_weights()
                for c, (c0, tc_sz) in enumerate(CHUNKS):
                    if c in xt1_pre:
                        xt = xt1_pre.pop(c)
                    else:
                        xt = xtp0.tile([128, tc_sz], BF16, tag="xt1",
                                       name=f"xt1_{c}", bufs=8)
                        nc.sync.dma_start(xt[:], xT_d[1][:, c0:c0 + tc_sz])
                    h1 = w2p.tile([128, 4, tc_sz], BF16, tag="h1",
                                  name=f"h1_{c}_1")
                    for m in range(4):
                        pp = ps_p1.tile([128, tc_sz], F32, tag="p1",
                                        name=f"p1b_{c}_{m}")
                        nc.tensor.matmul(pp[:], w1_bf[1][:, ts(m, 128)],
                                         xt[:], start=True, stop=True)
                        nc.scalar.activation(
                            h1[:, m, :], pp[:], AF.Relu,
                            bias=shift_t[0][:, 1, m:m + 1],
                            scale=scale_t[0][:, 1, m:m + 1])
                    if c == C - 1:
                        nc.scalar.mul(h1[:, :, PAD0:], h1[:, :, PAD0:], 0.0)
                    spl = w2p.tile([128, 4, tc_sz], BF16, tag="spl",
                                   name=f"spl_{c}_1", bufs=4)
                    for m in range(4):
                        pq = ps_p2.tile([128, tc_sz], F32, tag="p2",
                                        name=f"p2_{c}_1_{m}")
                        for k in range(4):
                            nc.tensor.matmul(pq[:], w2_t[1][:, k, ts(m, 128)],
                                             h1[:, k, :],
                                             start=(k == 0), stop=(k == 3))
                        if m % 2 == 0 and c < 20:
                            nc.scalar.copy(spl[:, m, :], pq[:])
                        else:
                            nc.vector.tensor_copy(spl[:, m, :], pq[:])
                    for m in range(4):
                        nc.vector.bn_stats(st2[:, 1, m, c, :], spl[:, m, :])
                    nc.sync.dma_start(spill[:, 1, :, c0:c0 + tc_sz], spl[:])
                    if c == 6:
                        finish_stats(1, 0)
                    if c == 10:
                        # prologue h2 for the first two chunks on the idle
                        # GPSIMD: ready long before pass-3, so their fused-M
                        # matmuls fire the moment the PE queue reaches them
                        for cp in range(2):
                            _, psz = CHUNKS[cp]
                            h2p = stat.tile([128, 4, psz], BF16,
                                            tag=f"h2p_{cp}", name=f"h2p_{cp}")
                            for k in range(4):
                                nc.gpsimd.tensor_scalar(
                                    h2p[:, k, :], pf3_tiles[cp][:, k, :],
                                    scale_t[1][:, 0, k:k + 1],
                                    shift_t[1][:, 0, k:k + 1],
                                    ALU.mult, ALU.add)
                                nc.gpsimd.tensor_scalar_max(
                                    h2p[:, k, :], h2p[:, k, :], 0.0)
                            h2p_tiles[cp] = h2p

                agg1 = stat.tile([128, 4, 2], F32, tag="agg", name="agg_1")
                for m in range(4):
                    nc.vector.bn_aggr(agg1[:, m, :], st2[:, 1, m, :, :])
                tmp21 = stat.tile([128, 4], F32, tag="tmp2", name="tmp2_1")
                nc.vector.tensor_scalar_mul(pay[1][1][:, :, 0], agg1[:, :, 0],
                                            float(NP))
                nc.vector.tensor_mul(tmp21[:], agg1[:, :, 0], agg1[:, :, 0])
                nc.vector.tensor_add(tmp21[:], tmp21[:], agg1[:, :, 1])
                nc.vector.tensor_scalar_mul(pay[1][1][:, :, 1], tmp21[:],
                                            float(NP))
                issue_allreduce(1, 1)

            w2p.release()
            parkp.release()
            xtp0.release()
            ps_p2.release()
            ps_p1.release()

            # ================= pass 3: BN2 -> fused M -> head ============
            PRO3 = 6  # chunks whose b0-half runs ahead of the last AllReduce
            with (
                tc.tile_pool(name="w3p", bufs=2) as w3p,
                tc.tile_pool(name="tpark", bufs=1) as tpark,
            ):
                def load_pre2(c, b):
                    c0, tc_sz = CHUNKS[c]
                    t = w3p.tile([128, 4, tc_sz], BF16, tag=f"pre2_{b}",
                                 bufs=3, name=f"pre2_{c}_{b}")
                    nc.sync.dma_start(t[:], spill[:, b, :, c0:c0 + tc_sz])
                    return t

                def h2_act(c, b, pre2, eng=None):
                    """branch-0 on ACT (1 op/slab); branch-1 on DVE (2 ops)."""
                    c0, tc_sz = CHUNKS[c]
                    h2 = w3p.tile([128, 4, tc_sz], BF16, tag=f"h2_{b}",
                                  bufs=3, name=f"h2_{c}_{b}")
                    for k in range(4):
                        if eng is not None:
                            eng.tensor_scalar(
                                h2[:, k, :], pre2[:, k, :],
                                scale_t[1][:, b, k:k + 1],
                                shift_t[1][:, b, k:k + 1],
                                ALU.mult, ALU.add)
                            eng.tensor_scalar_max(h2[:, k, :],
                                                  h2[:, k, :], 0.0)
                        elif b == 0:
                            nc.scalar.activation(
                                h2[:, k, :], pre2[:, k, :], AF.Relu,
                                bias=shift_t[1][:, 0, k:k + 1],
                                scale=scale_t[1][:, 0, k:k + 1])
                        else:
                            nc.vector.tensor_scalar(
                                h2[:, k, :], pre2[:, k, :],
                                scale_t[1][:, 1, k:k + 1],
                                shift_t[1][:, 1, k:k + 1],
                                ALU.mult, ALU.add)
                            nc.vector.tensor_scalar_max(h2[:, k, :],
                                                        h2[:, k, :], 0.0)
                    return h2

                def head(c, t_sb):
                    c0, tc_sz = CHUNKS[c]
                    po = ps_o.tile([10, tc_sz], F32, tag="o", name=f"po_{c}")
                    for k in range(4):
                        nc.tensor.matmul(po[:], wh2_t[:, k, :], t_sb[:, k, :],
                                         start=(k == 0), stop=(k == 3))
                    o_sb = w3p.tile([10, tc_sz], F32, tag="o_sb", name=f"o_sb_{c}")
                    nc.scalar.activation(o_sb[:], po[:], AF.Identity,
                                         bias=bh2_sb[:, 0:1])
                    nc.sync.dma_start(outd[:, c0:c0 + tc_sz], o_sb[:])

                # prologue: b0-halves of the first PRO3 chunks run while
                # AllReduce(1,1) is still in flight; partials parked as bf16
                parked_t = {}
                for c in range(PRO3):
                    c0, tc_sz = CHUNKS[c]
                    if c in h2p_tiles:
                        pf3_tiles.pop(c)
                        h2_0 = h2p_tiles.pop(c)
                    else:
                        pre2_0 = pf3_tiles.pop(c) if c in pf3_tiles else \
                            load_pre2(c, 0)
                        h2_0 = h2_act(c, 0, pre2_0)
                    tp = tpark.tile([128, 4, tc_sz], BF16, tag=f"tp_{c}",
                                    name=f"tp_{c}")
                    parked_t[c] = tp
                    for m in range(4):
                        ptl = ps_t.tile([128, tc_sz], F32, tag="t",
                                        name=f"ptl_{c}_{m}a")
                        for k in range(4):
                            nc.tensor.matmul(ptl[:], m_t[0][:, k, ts(m, 128)],
                                             h2_0[:, k, :],
                                             start=(k == 0), stop=(k == 3))
                        if m % 2 == 0:
                            nc.scalar.copy(tp[:, m, :], ptl[:])
                        else:
                            nc.vector.tensor_copy(tp[:, m, :], ptl[:])

                finish_stats(1, 1)

                # combine stage for prologue chunks (needs stats(1,1))
                for c in range(PRO3):
                    c0, tc_sz = CHUNKS[c]
                    h2_1 = h2_act(c, 1, load_pre2(c, 1))
                    t_sb = w3p.tile([128, 4, tc_sz], BF16, tag="t_sb", bufs=2,
                                    name=f"t_sb_{c}")
                    tp = parked_t.pop(c)
                    for m in range(4):
                        ptl = ps_t.tile([128, tc_sz], F32, tag="t",
                                        name=f"ptl_{c}_{m}b")
                        for k in range(4):
                            nc.tensor.matmul(ptl[:], m_t[1][:, k, ts(m, 128)],
                                             h2_1[:, k, :],
                                             start=(k == 0), stop=(k == 3))
                        # t_pre = (psum + bias) + parked, then relu
                        nc.vector.scalar_tensor_tensor(
                            t_sb[:, m, :], ptl[:], bp_sb[:, m:m + 1],
                            tp[:, m, :], ALU.add, ALU.add)
                    t_sb2 = w3p.tile([128, 4, tc_sz], BF16, tag="t_sb2", bufs=2,
                                     name=f"t_sb2_{c}")
                    nc.scalar.activation(t_sb2[:, :, :], t_sb[:, :, :], AF.Relu)
                    head(c, t_sb2)

                # steady state: full chunk in one pass
                for c in range(PRO3, C):
                    c0, tc_sz = CHUNKS[c]
                    h2_0 = h2_act(c, 0, load_pre2(c, 0))
                    h2_1 = h2_act(c, 1, load_pre2(c, 1))
                    t_sb = w3p.tile([128, 4, tc_sz], BF16, tag="t_sb", bufs=2,
                                    name=f"t_sb_{c}")
                    for m in range(4):
                        ptl = ps_t.tile([128, tc_sz], F32, tag="t",
                                        name=f"ptl_{c}_{m}")
                        for k in range(4):
                            nc.tensor.matmul(ptl[:], m_t[0][:, k, ts(m, 128)],
                                             h2_0[:, k, :],
                                             start=(k == 0), stop=False)
                        for k in range(4):
                            nc.tensor.matmul(ptl[:], m_t[1][:, k, ts(m, 128)],
                                             h2_1[:, k, :],
                                             start=False, stop=(k == 3))
                        nc.scalar.activation(t_sb[:, m, :], ptl[:], AF.Relu,
                                             bias=bp_sb[:, m:m + 1])
                    head(c, t_sb)

            ps_o.release()
            ps_t.release()

    nc.compile()
    return nc


def _get_program():
    if "nc" not in _CACHE:
        _CACHE["nc"] = _build_program()
    return _CACHE["nc"]


def kernel(**inputs):
    import ml_dtypes

    nc = _get_program()
    bf16 = ml_dtypes.bfloat16

    def shard_x(x):
        x = np.ascontiguousarray(x, dtype=np.float32).reshape(NCORES, NSH, 128)
        pad = np.zeros((NCORES, NP - NSH, 128), dtype=np.float32)
        return np.concatenate([x, pad], axis=1)  # [NCORES, NP, 128]

    xs = [shard_x(inputs["x_1"]), shard_x(inputs["x_2"])]
    # feature-major + gram layouts, bf16
    xT = [np.ascontiguousarray(x.transpose(0, 2, 1)).astype(bf16) for x in xs]
    xg = [np.ascontiguousarray(x.reshape(NCORES, 128, NSUB, 128)).astype(bf16)
          for x in xs]

    def km(w):  # [512, O] -> [128, 4, O] (contraction-major for lhsT slabs)
        O = w.shape[1]
        return np.ascontiguousarray(
            w.reshape(4, 128, O).transpose(1, 0, 2))

    def vec(v):  # [512] -> [128, 4]
        return np.ascontiguousarray(v.reshape(4, 128).T)

    f64 = np.float64
    Wh1 = np.asarray(inputs["Wh1"], f64)
    rep = {}
    for b, sfx in ((0, "1"), (1, "2")):
        rep[f"W1_{b}"] = np.asarray(inputs[f"W1_{sfx}"], np.float32).astype(bf16)
        rep[f"W2_{b}"] = km(np.asarray(inputs[f"W2_{sfx}"], np.float32)).astype(bf16)
        M = np.asarray(inputs[f"Wf_{sfx}"], f64) @ Wh1[b * 512:(b + 1) * 512, :]
        rep[f"M_{b}"] = km(M.astype(np.float32)).astype(bf16)
        for l, nm in ((0, "1"), (1, "2")):
            rep[f"g{l}_{b}"] = vec(np.asarray(inputs[f"g{nm}_{sfx}"], np.float32))
            rep[f"be{l}_{b}"] = vec(np.asarray(inputs[f"be{nm}_{sfx}"], np.float32))
    bp = (np.asarray(inputs["bf_1"], f64) @ Wh1[:512, :]
          + np.asarray(inputs["bf_2"], f64) @ Wh1[512:, :]
          + np.asarray(inputs["bh1"], f64))
    rep["BP"] = vec(bp.astype(np.float32))
    rep["WH2"] = km(np.asarray(inputs["Wh2"], np.float32)).astype(bf16)
    rep["BH2"] = np.ascontiguousarray(
        np.asarray(inputs["bh2"], np.float32).reshape(10, 1))
    aux = np.ones((128, 5), dtype=np.float32)
    aux[:, 4] = EPS
    rep["AUX"] = aux

    in_maps = []
    for c in range(NCORES):
        m = {"xT_0": xT[0][c], "xT_1": xT[1][c],
             "xg_0": xg[0][c], "xg_1": xg[1][c]}
        m.update(rep)
        in_maps.append(m)

    res = bass_utils.run_bass_kernel_spmd(nc, in_maps, core_ids=list(range(NCORES)))
    parts = [res.results[c]["OUT"][:, :NSH] for c in range(NCORES)]
    out = np.concatenate(parts, axis=1).T
    return np.ascontiguousarray(out, dtype=np.float32)
